# revision 6
# baseline (speedup 1.0000x reference)
"""Conv2d 3x3 (stride 1, pad 1) Trainium2 Bass kernel — Winograd F(2,3) along H.

Problem: x (32, 128, 56, 56) fp32, kernels (256, 128, 3, 3) fp32, b (256,) fp32
-> out (32, 256, 56, 56) fp32.

Strategy:
  - Data-parallel over batch: 32 images / 8 cores = 4 images per core. SPMD,
    no collectives.
  - Winograd F(2,3) along H (row pairs): the 3 kh taps collapse into 4
    components m_i = sum_kw U[i,kw]^T V_i(col-shift kw), so each pair of
    output rows costs 12 matmuls instead of 18 direct taps (2/3 the PE work;
    direct conv is PE-roofline-bound at ~92us/core, Winograd ~62us).
      V0 = x[2g-1]-x[2g+1], V1 = x[2g]+x[2g+1], V2 = x[2g+1]-x[2g],
      V3 = x[2g]-x[2g+2]
      y[2g]   = m0+m1+m2+b,  y[2g+1] = m1-m2-m3+b
  - U = G-transform of the kernel along kh (host, fp32) stored bf16 as 12
    [128cin x 256cout] blocks ordered (comp, kw=1,0,2).
  - Tiling: 7 row-pair groups per tile; all 4 m_i of a tile-half live in ONE
    4-bank PSUM tile [128, 4, 512] so a single ScalarE activation evicts all
    of them (PSUM->SBUF bf16, FD=1568) — scalar stays under the PE's ~2us
    per tile-half. Column zero-pad is implicit via ragged kw=0/2 windows
    (kw=1 first with start=True).
  - Output transform on DVE only (GpSimd elementwise contends with DVE for
    the shared SBUF port — measured 3x slowdowns — so GpSimd only issues two
    early DMAs): p/u = s1+-s2 batched across both cout halves (bf16 2x),
    then per half y_even = (s0+b)+p and y_odd = (u+b)-s3 as fused
    scalar_tensor_tensor ops writing interleaved rows of the fp32 y tile.
  - V transform (DVE bf16 2x, row pairs as outer AP dims): whole-image ops
    for images 1..3 (x prefetched one image ahead); image 0 is chunked
    per-tile (16-row halo chunks) so the first matmul is gated only on a
    229KB DMA + one DVE op, with the weight DMAs interleaved between chunk
    loads on the scalar queue and 5 warm-up matmuls (~1.6us) bridging the
    HAM clock gate into the first real matmul.
"""

import numpy as np
import ml_dtypes

import concourse.bass as bass
import concourse.tile as tile
from concourse import bacc, mybir
from concourse.bass_utils import run_bass_kernel_spmd

N_CORES = 8
N_FULL = 32
N_PER = N_FULL // N_CORES  # 4 images per core
C_IN = 128
C_OUT = 256
H = W = 56
G = H // 2          # 28 row-pair groups
TK = 4              # m-tiles per image (along H)
GPT = G // TK       # 7 row-pair groups per tile
FD = GPT * W        # 392

_DT = mybir.dt.bfloat16
_F32 = mybir.dt.float32
_ID = mybir.ActivationFunctionType.Identity
_ADD = mybir.AluOpType.add
_SUB = mybir.AluOpType.subtract

_KW_ORDER = [1, 0, 2]
_KW_POS = {1: 0, 0: 1, 2: 2}


def _build():
    nc = bacc.Bacc(
        "TRN2",
        target_bir_lowering=False,
        debug=False,
        num_devices=N_CORES,
    )
    xs = nc.dram_tensor("xs", [N_PER, C_IN, H, W], _DT, kind="ExternalInput").ap()
    wt = nc.dram_tensor("wt", [C_IN, 12 * C_OUT], _DT, kind="ExternalInput").ap()
    bt = nc.dram_tensor("bt", [128, 2], _F32, kind="ExternalInput").ap()
    y = nc.dram_tensor("y", [N_PER, C_OUT, H, W], _F32, kind="ExternalOutput").ap()

    with tile.TileContext(nc) as tc:
        with (
            tc.tile_pool(name="const", bufs=1) as const,
            tc.tile_pool(name="xcpool", bufs=4) as xcpool,
            tc.tile_pool(name="xipool", bufs=2) as xipool,
            tc.tile_pool(name="vpool", bufs=8) as vpool,
            tc.tile_pool(name="spool", bufs=3) as spool,
            tc.tile_pool(name="qpool", bufs=6) as qpool,
            tc.tile_pool(name="ypool", bufs=4) as ypool,
            tc.tile_pool(name="pspool", bufs=2, space="PSUM") as pspool,
        ):
            # PE warm-up: ~1.6us of dummy matmuls during the input-load
            # window keep the HAM activity window busy so the clock gate
            # lifts (1.2 -> 2.4 GHz) right as the real matmuls begin.
            warm = const.tile([128, 512], _DT)
            nc.vector.memset(warm[:], 0.0)
            wps = pspool.tile([128, 4, 512], _F32, tag="ps", name="warm_ps")
            N_WARM = 5
            for i in range(N_WARM):
                nc.tensor.matmul(
                    wps[:, 0, :FD],
                    lhsT=warm[:, :128],
                    rhs=warm[:, :FD],
                    start=(i == 0),
                    stop=(i == N_WARM - 1),
                )

            bias_sb = const.tile([128, 2], _F32)
            nc.scalar.dma_start(out=bias_sb[:], in_=bt)
            wt_sb0 = const.tile([C_IN, 6 * C_OUT], _DT)
            wt_sb1 = const.tile([C_IN, 6 * C_OUT], _DT)

            def wslice(i, kw, half):
                bi = i * 3 + _KW_POS[kw]
                sb = wt_sb0 if bi < 6 else wt_sb1
                c0 = (bi % 6) * C_OUT + half * 128
                return sb[:, c0 : c0 + 128]

            # ---- image 0: chunked x + per-chunk V, weights interleaved ----
            v0tiles = [
                vpool.tile([C_IN, G, W], _DT, tag="v", name=f"v{i}_0")
                for i in range(4)
            ]
            xc_tiles = []
            for k in range(TK):
                lo = max(0, 14 * k - 1)
                hi = min(H, 14 * k + 15)
                loff = lo - (14 * k - 1)
                xc = xcpool.tile([C_IN, 16, W], _DT, tag="xc", name=f"xc{k}")
                nc.scalar.dma_start(
                    out=xc[:, loff : loff + (hi - lo), :], in_=xs[0, :, lo:hi, :]
                )
                xc_tiles.append(xc)
                if k == 0:
                    # weight part A (comp 0) right behind chunk 0
                    nc.scalar.dma_start(out=wt_sb0[:, : 3 * C_OUT], in_=wt[:, : 3 * C_OUT])
                elif k == 1:
                    nc.scalar.dma_start(
                        out=wt_sb0[:, 3 * C_OUT :], in_=wt[:, 3 * C_OUT : 6 * C_OUT]
                    )
                    # comps 2-3 + image-1 prefetch ride the gpsimd queue
                    nc.gpsimd.dma_start(out=wt_sb1[:], in_=wt[:, 6 * C_OUT :])

                v = v0tiles
                g0 = 7 * k
                if k == 0:
                    nc.vector.tensor_sub(
                        v[0][:, 1:7, :], xc[:, 2:14:2, :], xc[:, 4:16:2, :]
                    )
                    nc.vector.tensor_scalar_mul(v[0][:, 0:1, :], xc[:, 2:3, :], -1.0)
                else:
                    nc.vector.tensor_sub(
                        v[0][:, g0 : g0 + 7, :], xc[:, 0:14:2, :], xc[:, 2:16:2, :]
                    )
                nc.vector.tensor_add(
                    v[1][:, g0 : g0 + 7, :], xc[:, 1:15:2, :], xc[:, 2:16:2, :]
                )
                nc.vector.tensor_sub(
                    v[2][:, g0 : g0 + 7, :], xc[:, 2:16:2, :], xc[:, 1:15:2, :]
                )
                if k == TK - 1:
                    nc.vector.tensor_sub(
                        v[3][:, g0 : g0 + 6, :], xc[:, 1:13:2, :], xc[:, 3:15:2, :]
                    )
                    nc.vector.tensor_copy(v[3][:, G - 1 : G, :], xc[:, 13:14, :])
                else:
                    nc.vector.tensor_sub(
                        v[3][:, g0 : g0 + 7, :], xc[:, 1:15:2, :], xc[:, 3:16:2, :]
                    )

            xi_tiles = {}

            def load_xi(n, eng):
                xi = xipool.tile([C_IN, H, W], _DT, tag="xi", name=f"xi{n}")
                eng.dma_start(out=xi[:], in_=xs[n])
                xi_tiles[n] = xi

            load_xi(1, nc.gpsimd)

            def v_transform(n):
                xi = xi_tiles[n]
                v = [
                    vpool.tile([C_IN, G, W], _DT, tag="v", name=f"v{i}_{n}")
                    for i in range(4)
                ]
                nc.vector.tensor_sub(
                    v[0][:, 1:G, :], xi[:, 1:54:2, :], xi[:, 3:56:2, :]
                )
                nc.vector.tensor_scalar_mul(v[0][:, 0:1, :], xi[:, 1:2, :], -1.0)
                nc.vector.tensor_add(v[1][:], xi[:, 0:55:2, :], xi[:, 1:56:2, :])
                nc.vector.tensor_sub(v[2][:], xi[:, 1:56:2, :], xi[:, 0:55:2, :])
                nc.vector.tensor_sub(
                    v[3][:, 0 : G - 1, :], xi[:, 0:53:2, :], xi[:, 2:55:2, :]
                )
                nc.vector.tensor_copy(v[3][:, G - 1 : G, :], xi[:, 54:55, :])
                return v

            def as3(ap):
                return ap.rearrange("p (g w) -> p g w", g=GPT)

            for n in range(N_PER):
                v = v0tiles if n == 0 else v_transform(n)
                for k in range(TK):
                    g0 = 7 * k
                    st = spool.tile([128, 2, 4, FD], _DT, tag="st", name=f"st_{n}_{k}")
                    for half in range(2):
                        m = pspool.tile(
                            [128, 4, 512], _F32, tag="ps", name=f"m_{n}_{k}_{half}"
                        )
                        for i in range(4):
                            mi = m[:, i, :FD].rearrange("p (g w) -> p g w", g=GPT)
                            for kw in _KW_ORDER:
                                if kw == 1:
                                    out_ap = mi
                                    rhs = v[i][:, g0 : g0 + 7, :]
                                elif kw == 0:
                                    out_ap = mi[:, :, 1:W]
                                    rhs = v[i][:, g0 : g0 + 7, 0 : W - 1]
                                else:
                                    out_ap = mi[:, :, 0 : W - 1]
                                    rhs = v[i][:, g0 : g0 + 7, 1:W]
                                nc.tensor.matmul(
                                    out_ap,
                                    lhsT=wslice(i, kw, half),
                                    rhs=rhs,
                                    start=(kw == 1),
                                    stop=(kw == 2),
                                )
                        # one act evicts all four comps (bf16, FD=1568)
                        nc.scalar.activation(st[:, half], m[:, :, :FD], _ID)

                    # prefetch the image after next behind the first tile
                    if k == 0 and n + 2 < N_PER:
                        load_xi(n + 2, nc.sync)

                    # output transform; p/u batched across both cout halves
                    pb = qpool.tile([128, 2, FD], _DT, tag="pb", name=f"pb_{n}_{k}")
                    nc.vector.tensor_add(pb[:], st[:, :, 1, :], st[:, :, 2, :])
                    ub = qpool.tile([128, 2, FD], _DT, tag="ub", name=f"ub_{n}_{k}")
                    nc.vector.tensor_sub(ub[:], st[:, :, 1, :], st[:, :, 2, :])

                    for half in range(2):
                        bias_ap = bias_sb[:, half : half + 1]
                        yt = ypool.tile(
                            [128, 14, W], _F32, tag="yt", name=f"y_{n}_{k}_{half}"
                        )
                        nc.vector.scalar_tensor_tensor(
                            yt[:, 0:14:2, :],
                            as3(st[:, half, 0, :]),
                            bias_ap,
                            as3(pb[:, half, :]),
                            op0=_ADD,
                            op1=_ADD,
                        )
                        nc.vector.scalar_tensor_tensor(
                            yt[:, 1:14:2, :],
                            as3(ub[:, half, :]),
                            bias_ap,
                            as3(st[:, half, 3, :]),
                            op0=_ADD,
                            op1=_SUB,
                        )

                        y_slice = y[
                            n, half * 128 : (half + 1) * 128, 14 * k : 14 * k + 14, :
                        ]
                        if n == N_PER - 1 and half == 1 and k == TK - 1:
                            # split the final store so its DMA drain doesn't
                            # gate the end barrier on one queue
                            nc.sync.dma_start(out=y_slice[:, 0:7, :], in_=yt[:, 0:7, :])
                            nc.scalar.dma_start(
                                out=y_slice[:, 7:14, :], in_=yt[:, 7:14, :]
                            )
                        else:
                            nc.sync.dma_start(out=y_slice, in_=yt[:])
    nc.compile()
    return nc


_NC = None


def _get_nc():
    global _NC
    if _NC is None:
        _NC = _build()
    return _NC


def _prep_inputs(x, kernels, b):
    bf16 = ml_dtypes.bfloat16
    xb = np.ascontiguousarray(x, dtype=np.float32).astype(bf16)
    w = np.asarray(kernels, dtype=np.float32)  # [O, C, kh, kw]
    U = [
        w[:, :, 0, :],
        0.5 * (w[:, :, 0, :] + w[:, :, 1, :] + w[:, :, 2, :]),
        0.5 * (w[:, :, 0, :] - w[:, :, 1, :] + w[:, :, 2, :]),
        w[:, :, 2, :],
    ]
    blocks = []
    for Ui in U:
        for kw in _KW_ORDER:
            blocks.append(Ui[:, :, kw].T)  # [C=128, O=256]
    wtb = np.ascontiguousarray(np.concatenate(blocks, axis=1)).astype(bf16)
    # bias [256] -> [128, 2]: column h holds b[h*128 : (h+1)*128]
    btb = np.ascontiguousarray(np.asarray(b, dtype=np.float32).reshape(2, 128).T)
    return xb, wtb, btb


def kernel(x, kernels, b):
    nc = _get_nc()
    xb, wtb, btb = _prep_inputs(x, kernels, b)
    in_maps = [
        {"xs": xb[i * N_PER : (i + 1) * N_PER], "wt": wtb, "bt": btb}
        for i in range(N_CORES)
    ]
    res = run_bass_kernel_spmd(nc, in_maps, core_ids=list(range(N_CORES)))
    out = np.concatenate(
        [r["y"].reshape(N_PER, C_OUT, H, W) for r in res.results], axis=0
    )
    return np.ascontiguousarray(out, dtype=np.float32)


# revision 10
# speedup vs baseline: 1.0442x; 1.0442x over previous
"""Conv2d 3x3 (stride 1, pad 1) Trainium2 Bass kernel — Winograd F(2,3) along H.

Problem: x (32, 128, 56, 56) fp32, kernels (256, 128, 3, 3) fp32, b (256,) fp32
-> out (32, 256, 56, 56) fp32.

Strategy:
  - Data-parallel over batch: 32 images / 8 cores = 4 images per core. SPMD,
    no collectives.
  - Winograd F(2,3) along H (row pairs): the 3 kh taps collapse into 4
    components m_i = sum_kw U[i,kw]^T V_i(col-shift kw), so each pair of
    output rows costs 12 matmuls instead of 18 direct taps (2/3 the PE work;
    direct conv is PE-roofline-bound at ~92us/core, Winograd ~62us).
      V0 = x[2g-1]-x[2g+1], V1 = x[2g]+x[2g+1], V2 = x[2g+1]-x[2g],
      V3 = x[2g]-x[2g+2]
      y[2g]   = m0+m1+m2+b,  y[2g+1] = m1-m2-m3+b
  - U = G-transform of the kernel along kh (host, fp32) stored bf16 as 12
    [128cin x 256cout] blocks ordered (comp, kw=1,0,2).
  - Tiling: 7 row-pair groups per tile; all 4 m_i of a tile-half live in ONE
    4-bank PSUM tile [128, 4, 512] so a single ScalarE activation evicts all
    of them (PSUM->SBUF bf16, FD=1568) — scalar stays under the PE's ~2us
    per tile-half. Column zero-pad is implicit via ragged kw=0/2 windows
    (kw=1 first with start=True).
  - Output transform on DVE only (GpSimd elementwise contends with DVE for
    the shared SBUF port — measured 3x slowdowns — so GpSimd only issues two
    early DMAs): p/u = s1+-s2 batched across both cout halves (bf16 2x),
    then per half y_even = (s0+b)+p and y_odd = (u+b)-s3 as fused
    scalar_tensor_tensor ops writing interleaved rows of the fp32 y tile.
  - V transform (DVE bf16 2x, row pairs as outer AP dims): whole-image ops
    for images 1..3 (x prefetched one image ahead); image 0 is chunked
    per-tile (16-row halo chunks) so the first matmul is gated only on a
    229KB DMA + one DVE op, with the weight DMAs interleaved between chunk
    loads on the scalar queue and 5 warm-up matmuls (~1.6us) bridging the
    HAM clock gate into the first real matmul.
"""

import numpy as np
import ml_dtypes

import concourse.bass as bass
import concourse.tile as tile
from concourse import bacc, mybir
from concourse.bass_utils import run_bass_kernel_spmd

N_CORES = 8
N_FULL = 32
N_PER = N_FULL // N_CORES  # 4 images per core
C_IN = 128
C_OUT = 256
H = W = 56
G = H // 2          # 28 row-pair groups
TK = 4              # m-tiles per image (along H)
GPT = G // TK       # 7 row-pair groups per tile
FD = GPT * W        # 392

_DT = mybir.dt.bfloat16
_F32 = mybir.dt.float32
_ID = mybir.ActivationFunctionType.Identity
_ADD = mybir.AluOpType.add
_SUB = mybir.AluOpType.subtract

_KW_ORDER = [1, 0, 2]
_KW_POS = {1: 0, 0: 1, 2: 2}


def _build():
    nc = bacc.Bacc(
        "TRN2",
        target_bir_lowering=False,
        debug=False,
        num_devices=N_CORES,
    )
    xs = nc.dram_tensor("xs", [N_PER, C_IN, H, W], _DT, kind="ExternalInput").ap()
    wt = nc.dram_tensor("wt", [C_IN, 12 * C_OUT], _DT, kind="ExternalInput").ap()
    bt = nc.dram_tensor("bt", [128, 2], _F32, kind="ExternalInput").ap()
    y = nc.dram_tensor("y", [N_PER, C_OUT, H, W], _F32, kind="ExternalOutput").ap()

    with tile.TileContext(nc) as tc:
        with (
            tc.tile_pool(name="const", bufs=1) as const,
            tc.tile_pool(name="xcpool", bufs=4) as xcpool,
            tc.tile_pool(name="xipool", bufs=2) as xipool,
            tc.tile_pool(name="vpool", bufs=8) as vpool,
            tc.tile_pool(name="spool", bufs=3) as spool,
            tc.tile_pool(name="qpool", bufs=6) as qpool,
            tc.tile_pool(name="ypool", bufs=4) as ypool,
            tc.tile_pool(name="pspool", bufs=2, space="PSUM") as pspool,
        ):
            # PE warm-up: ~1.6us of dummy matmuls during the input-load
            # window keep the HAM activity window busy so the clock gate
            # lifts (1.2 -> 2.4 GHz) right as the real matmuls begin.
            warm = const.tile([128, 512], _DT)
            nc.vector.memset(warm[:], 0.0)
            wps = pspool.tile([128, 4, 512], _F32, tag="ps", name="warm_ps")
            N_WARM = 5
            for i in range(N_WARM):
                nc.tensor.matmul(
                    wps[:, 0, :FD],
                    lhsT=warm[:, :128],
                    rhs=warm[:, :FD],
                    start=(i == 0),
                    stop=(i == N_WARM - 1),
                )

            bias_sb = const.tile([128, 2], _F32)
            wt_sb0 = const.tile([C_IN, 6 * C_OUT], _DT)
            wt_sb1 = const.tile([C_IN, 6 * C_OUT], _DT)

            def wslice(i, kw, half):
                bi = i * 3 + _KW_POS[kw]
                sb = wt_sb0 if bi < 6 else wt_sb1
                c0 = (bi % 6) * C_OUT + half * 128
                return sb[:, c0 : c0 + 128]

            # ---- image 0: chunked x + per-chunk V, weights interleaved ----
            v0tiles = [
                vpool.tile([C_IN, G, W], _DT, tag="v", name=f"v{i}_0")
                for i in range(4)
            ]
            xc_tiles = []
            for k in range(TK):
                lo = max(0, 14 * k - 1)
                hi = min(H, 14 * k + 15)
                loff = lo - (14 * k - 1)
                xc = xcpool.tile([C_IN, 16, W], _DT, tag="xc", name=f"xc{k}")
                nc.scalar.dma_start(
                    out=xc[:, loff : loff + (hi - lo), :], in_=xs[0, :, lo:hi, :]
                )
                xc_tiles.append(xc)
                if k == 0:
                    # weight part A (comp 0) right behind chunk 0; the rest
                    # of the weights on the gpsimd queue in parallel
                    nc.scalar.dma_start(out=wt_sb0[:, : 3 * C_OUT], in_=wt[:, : 3 * C_OUT])
                    nc.gpsimd.dma_start(out=wt_sb1[:], in_=wt[:, 6 * C_OUT :])
                elif k == 1:
                    nc.scalar.dma_start(
                        out=wt_sb0[:, 3 * C_OUT :], in_=wt[:, 3 * C_OUT : 6 * C_OUT]
                    )
                elif k == 3:
                    nc.scalar.dma_start(out=bias_sb[:], in_=bt)

                v = v0tiles
                g0 = 7 * k
                if k == 0:
                    nc.vector.tensor_sub(
                        v[0][:, 1:7, :], xc[:, 2:14:2, :], xc[:, 4:16:2, :]
                    )
                    nc.vector.tensor_scalar_mul(v[0][:, 0:1, :], xc[:, 2:3, :], -1.0)
                else:
                    nc.vector.tensor_sub(
                        v[0][:, g0 : g0 + 7, :], xc[:, 0:14:2, :], xc[:, 2:16:2, :]
                    )
                nc.vector.tensor_add(
                    v[1][:, g0 : g0 + 7, :], xc[:, 1:15:2, :], xc[:, 2:16:2, :]
                )
                nc.vector.tensor_sub(
                    v[2][:, g0 : g0 + 7, :], xc[:, 2:16:2, :], xc[:, 1:15:2, :]
                )
                if k == TK - 1:
                    nc.vector.tensor_sub(
                        v[3][:, g0 : g0 + 6, :], xc[:, 1:13:2, :], xc[:, 3:15:2, :]
                    )
                    nc.vector.tensor_copy(v[3][:, G - 1 : G, :], xc[:, 13:14, :])
                else:
                    nc.vector.tensor_sub(
                        v[3][:, g0 : g0 + 7, :], xc[:, 1:15:2, :], xc[:, 3:16:2, :]
                    )

            xi_tiles = {}

            def load_xi(n, eng):
                xi = xipool.tile([C_IN, H, W], _DT, tag="xi", name=f"xi{n}")
                eng.dma_start(out=xi[:], in_=xs[n])
                xi_tiles[n] = xi

            load_xi(1, nc.gpsimd)

            def v_transform(n):
                xi = xi_tiles[n]
                v = [
                    vpool.tile([C_IN, G, W], _DT, tag="v", name=f"v{i}_{n}")
                    for i in range(4)
                ]
                nc.vector.tensor_sub(
                    v[0][:, 1:G, :], xi[:, 1:54:2, :], xi[:, 3:56:2, :]
                )
                nc.vector.tensor_scalar_mul(v[0][:, 0:1, :], xi[:, 1:2, :], -1.0)
                nc.vector.tensor_add(v[1][:], xi[:, 0:55:2, :], xi[:, 1:56:2, :])
                nc.vector.tensor_sub(v[2][:], xi[:, 1:56:2, :], xi[:, 0:55:2, :])
                nc.vector.tensor_sub(
                    v[3][:, 0 : G - 1, :], xi[:, 0:53:2, :], xi[:, 2:55:2, :]
                )
                nc.vector.tensor_copy(v[3][:, G - 1 : G, :], xi[:, 54:55, :])
                return v

            def as3(ap):
                return ap.rearrange("p (g w) -> p g w", g=GPT)

            for n in range(N_PER):
                v = v0tiles if n == 0 else v_transform(n)
                for k in range(TK):
                    g0 = 7 * k
                    last = n == N_PER - 1 and k == TK - 1
                    st = spool.tile([128, 2, 4, FD], _DT, tag="st", name=f"st_{n}_{k}")
                    for half in range(2):
                        m = pspool.tile(
                            [128, 4, 512], _F32, tag="ps", name=f"m_{n}_{k}_{half}"
                        )
                        for i in range(4):
                            mi = m[:, i, :FD].rearrange("p (g w) -> p g w", g=GPT)
                            for kw in _KW_ORDER:
                                if kw == 1:
                                    out_ap = mi
                                    rhs = v[i][:, g0 : g0 + 7, :]
                                elif kw == 0:
                                    out_ap = mi[:, :, 1:W]
                                    rhs = v[i][:, g0 : g0 + 7, 0 : W - 1]
                                else:
                                    out_ap = mi[:, :, 0 : W - 1]
                                    rhs = v[i][:, g0 : g0 + 7, 1:W]
                                nc.tensor.matmul(
                                    out_ap,
                                    lhsT=wslice(i, kw, half),
                                    rhs=rhs,
                                    start=(kw == 1),
                                    stop=(kw == 2),
                                )
                        if last:
                            # split the act so eviction overlaps the final
                            # matmuls and the drain chain is short
                            nc.scalar.activation(
                                st[:, half, 0:2], m[:, 0:2, :FD], _ID
                            )
                            nc.scalar.activation(
                                st[:, half, 2:4], m[:, 2:4, :FD], _ID
                            )
                        else:
                            # one act evicts all four comps (bf16, FD=1568)
                            nc.scalar.activation(st[:, half], m[:, :, :FD], _ID)

                    # prefetch the image after next; emitted at k=2 so the
                    # transfer queues behind this image's first y stores and
                    # cannot steal startup DMA bandwidth
                    if k == 2 and n + 2 < N_PER:
                        load_xi(n + 2, nc.sync)

                    # output transform; p/u batched across both cout halves
                    # (per-half for the last tile to shorten the tail chain)
                    pb = qpool.tile([128, 2, FD], _DT, tag="pb", name=f"pb_{n}_{k}")
                    ub = qpool.tile([128, 2, FD], _DT, tag="ub", name=f"ub_{n}_{k}")
                    if last:
                        for half in range(2):
                            nc.vector.tensor_add(
                                pb[:, half], st[:, half, 1, :], st[:, half, 2, :]
                            )
                            nc.vector.tensor_sub(
                                ub[:, half], st[:, half, 1, :], st[:, half, 2, :]
                            )
                    else:
                        nc.vector.tensor_add(pb[:], st[:, :, 1, :], st[:, :, 2, :])
                        nc.vector.tensor_sub(ub[:], st[:, :, 1, :], st[:, :, 2, :])

                    for half in range(2):
                        bias_ap = bias_sb[:, half : half + 1]
                        yt = ypool.tile(
                            [128, 14, W], _F32, tag="yt", name=f"y_{n}_{k}_{half}"
                        )
                        nc.vector.scalar_tensor_tensor(
                            yt[:, 0:14:2, :],
                            as3(st[:, half, 0, :]),
                            bias_ap,
                            as3(pb[:, half, :]),
                            op0=_ADD,
                            op1=_ADD,
                        )
                        nc.vector.scalar_tensor_tensor(
                            yt[:, 1:14:2, :],
                            as3(ub[:, half, :]),
                            bias_ap,
                            as3(st[:, half, 3, :]),
                            op0=_ADD,
                            op1=_SUB,
                        )

                        y_slice = y[
                            n, half * 128 : (half + 1) * 128, 14 * k : 14 * k + 14, :
                        ]
                        if last and half == 1:
                            # split the final store so its DMA drain doesn't
                            # gate the end barrier on one queue
                            nc.sync.dma_start(out=y_slice[:, 0:7, :], in_=yt[:, 0:7, :])
                            nc.scalar.dma_start(
                                out=y_slice[:, 7:14, :], in_=yt[:, 7:14, :]
                            )
                        elif n == N_PER - 1 and half == 1:
                            # spread the last image's stores over two queues
                            # to avoid a terminal backlog on sync
                            nc.scalar.dma_start(out=y_slice, in_=yt[:])
                        else:
                            nc.sync.dma_start(out=y_slice, in_=yt[:])
    nc.compile()
    return nc


_NC = None


def _get_nc():
    global _NC
    if _NC is None:
        _NC = _build()
    return _NC


def _prep_inputs(x, kernels, b):
    bf16 = ml_dtypes.bfloat16
    xb = np.ascontiguousarray(x, dtype=np.float32).astype(bf16)
    w = np.asarray(kernels, dtype=np.float32)  # [O, C, kh, kw]
    U = [
        w[:, :, 0, :],
        0.5 * (w[:, :, 0, :] + w[:, :, 1, :] + w[:, :, 2, :]),
        0.5 * (w[:, :, 0, :] - w[:, :, 1, :] + w[:, :, 2, :]),
        w[:, :, 2, :],
    ]
    blocks = []
    for Ui in U:
        for kw in _KW_ORDER:
            blocks.append(Ui[:, :, kw].T)  # [C=128, O=256]
    wtb = np.ascontiguousarray(np.concatenate(blocks, axis=1)).astype(bf16)
    # bias [256] -> [128, 2]: column h holds b[h*128 : (h+1)*128]
    btb = np.ascontiguousarray(np.asarray(b, dtype=np.float32).reshape(2, 128).T)
    return xb, wtb, btb


def kernel(x, kernels, b):
    nc = _get_nc()
    xb, wtb, btb = _prep_inputs(x, kernels, b)
    in_maps = [
        {"xs": xb[i * N_PER : (i + 1) * N_PER], "wt": wtb, "bt": btb}
        for i in range(N_CORES)
    ]
    res = run_bass_kernel_spmd(nc, in_maps, core_ids=list(range(N_CORES)))
    out = np.concatenate(
        [r["y"].reshape(N_PER, C_OUT, H, W) for r in res.results], axis=0
    )
    return np.ascontiguousarray(out, dtype=np.float32)


# revision 14
# speedup vs baseline: 1.0851x; 1.0392x over previous
"""Conv2d 3x3 (stride 1, pad 1) Trainium2 Bass kernel — Winograd F(2,3) along H.

Problem: x (32, 128, 56, 56) fp32, kernels (256, 128, 3, 3) fp32, b (256,) fp32
-> out (32, 256, 56, 56) fp32.

Strategy:
  - Data-parallel over batch: 32 images / 8 cores = 4 images per core. SPMD,
    no collectives.
  - Winograd F(2,3) along H (row pairs): the 3 kh taps collapse into 4
    components m_i = sum_kw U[i,kw]^T V_i(col-shift kw), so each pair of
    output rows costs 12 matmuls instead of 18 direct taps (2/3 the PE work;
    direct conv is PE-roofline-bound at ~92us/core, Winograd ~62us).
      V0 = x[2g-1]-x[2g+1], V1 = x[2g]+x[2g+1], V2 = x[2g+1]-x[2g],
      V3 = x[2g]-x[2g+2]
      y[2g]   = m0+m1+m2+b,  y[2g+1] = m1-m2-m3+b
  - U = G-transform of the kernel along kh (host, fp32) stored bf16 as 12
    [128cin x 256cout] blocks ordered (comp, kw=1,0,2).
  - Tiling: 7 row-pair groups per tile; all 4 m_i of a tile-half live in ONE
    4-bank PSUM tile [128, 4, 512] so a single ScalarE activation evicts all
    of them (PSUM->SBUF bf16, FD=1568) — scalar stays under the PE's ~2us
    per tile-half. Column zero-pad is implicit via ragged kw=0/2 windows
    (kw=1 first with start=True).
  - Output transform on DVE only (GpSimd elementwise contends with DVE for
    the shared SBUF port — measured 3x slowdowns — so GpSimd only issues two
    early DMAs): p/u = s1+-s2 batched across both cout halves (bf16 2x),
    then per half y_even = (s0+b)+p and y_odd = (u+b)-s3 as fused
    scalar_tensor_tensor ops writing interleaved rows of the fp32 y tile.
  - V transform (DVE bf16 2x, row pairs as outer AP dims): whole-image ops
    for images 1..3 (x prefetched one image ahead); image 0 is chunked
    per-tile (16-row halo chunks) so the first matmul is gated only on a
    229KB DMA + one DVE op, with the weight DMAs interleaved between chunk
    loads on the scalar queue and 5 warm-up matmuls (~1.6us) bridging the
    HAM clock gate into the first real matmul.
"""

import numpy as np
import ml_dtypes

import concourse.bass as bass
import concourse.tile as tile
from concourse import bacc, mybir
from concourse.bass_utils import run_bass_kernel_spmd

N_CORES = 8
N_FULL = 32
N_PER = N_FULL // N_CORES  # 4 images per core
C_IN = 128
C_OUT = 256
H = W = 56
G = H // 2          # 28 row-pair groups
TK = 4              # m-tiles per image (along H)
GPT = G // TK       # 7 row-pair groups per tile
FD = GPT * W        # 392

_DT = mybir.dt.bfloat16
_F32 = mybir.dt.float32
_ID = mybir.ActivationFunctionType.Identity
_ADD = mybir.AluOpType.add
_SUB = mybir.AluOpType.subtract

_KW_ORDER = [1, 0, 2]
_KW_POS = {1: 0, 0: 1, 2: 2}


def _build():
    nc = bacc.Bacc(
        "TRN2",
        target_bir_lowering=False,
        debug=False,
        num_devices=N_CORES,
    )
    xs = nc.dram_tensor("xs", [N_PER, C_IN, H, W], _DT, kind="ExternalInput").ap()
    wt = nc.dram_tensor("wt", [C_IN, 12 * C_OUT], _DT, kind="ExternalInput").ap()
    bt = nc.dram_tensor("bt", [128, 2], _F32, kind="ExternalInput").ap()
    y = nc.dram_tensor("y", [N_PER, C_OUT, H, W], _F32, kind="ExternalOutput").ap()

    with tile.TileContext(nc) as tc:
        with (
            tc.tile_pool(name="const", bufs=1) as const,
            tc.tile_pool(name="xcpool", bufs=4) as xcpool,
            tc.tile_pool(name="xipool", bufs=3) as xipool,
            tc.tile_pool(name="vpool", bufs=8) as vpool,
            tc.tile_pool(name="spool", bufs=3) as spool,
            tc.tile_pool(name="qpool", bufs=6) as qpool,
            tc.tile_pool(name="ypool", bufs=4) as ypool,
            tc.tile_pool(name="pspool", bufs=2, space="PSUM") as pspool,
        ):
            # PE warm-up: ~1.6us of dummy matmuls during the input-load
            # window keep the HAM activity window busy so the clock gate
            # lifts (1.2 -> 2.4 GHz) right as the real matmuls begin.
            warm = const.tile([128, 512], _DT)
            nc.vector.memset(warm[:], 0.0)
            wps = pspool.tile([128, 4, 512], _F32, tag="ps", name="warm_ps")
            N_WARM = 5
            for i in range(N_WARM):
                nc.tensor.matmul(
                    wps[:, 0, :FD],
                    lhsT=warm[:, :128],
                    rhs=warm[:, :FD],
                    start=(i == 0),
                    stop=(i == N_WARM - 1),
                )

            bias_sb = const.tile([128, 2], _F32)
            wt_sb0 = const.tile([C_IN, 6 * C_OUT], _DT)
            wt_sb1 = const.tile([C_IN, 6 * C_OUT], _DT)

            def wslice(i, kw, half):
                bi = i * 3 + _KW_POS[kw]
                sb = wt_sb0 if bi < 6 else wt_sb1
                c0 = (bi % 6) * C_OUT + half * 128
                return sb[:, c0 : c0 + 128]

            # ---- image 0: chunked x + per-chunk V, weights interleaved ----
            v0tiles = [
                vpool.tile([C_IN, G, W], _DT, tag="v", name=f"v{i}_0")
                for i in range(4)
            ]
            xc_tiles = []
            for k in range(TK):
                lo = max(0, 14 * k - 1)
                hi = min(H, 14 * k + 15)
                loff = lo - (14 * k - 1)
                xc = xcpool.tile([C_IN, 16, W], _DT, tag="xc", name=f"xc{k}")
                nc.scalar.dma_start(
                    out=xc[:, loff : loff + (hi - lo), :], in_=xs[0, :, lo:hi, :]
                )
                xc_tiles.append(xc)
                if k == 0:
                    # weight part A (comp 0) right behind chunk 0; the rest
                    # of the weights on the gpsimd queue in parallel
                    nc.scalar.dma_start(out=wt_sb0[:, : 3 * C_OUT], in_=wt[:, : 3 * C_OUT])
                    nc.gpsimd.dma_start(out=wt_sb1[:], in_=wt[:, 6 * C_OUT :])
                elif k == 1:
                    nc.scalar.dma_start(
                        out=wt_sb0[:, 3 * C_OUT :], in_=wt[:, 3 * C_OUT : 6 * C_OUT]
                    )

                v = v0tiles
                g0 = 7 * k
                if k == 0:
                    nc.vector.tensor_sub(
                        v[0][:, 1:7, :], xc[:, 2:14:2, :], xc[:, 4:16:2, :]
                    )
                    nc.vector.tensor_scalar_mul(v[0][:, 0:1, :], xc[:, 2:3, :], -1.0)
                else:
                    nc.vector.tensor_sub(
                        v[0][:, g0 : g0 + 7, :], xc[:, 0:14:2, :], xc[:, 2:16:2, :]
                    )
                nc.vector.tensor_add(
                    v[1][:, g0 : g0 + 7, :], xc[:, 1:15:2, :], xc[:, 2:16:2, :]
                )
                nc.vector.tensor_sub(
                    v[2][:, g0 : g0 + 7, :], xc[:, 2:16:2, :], xc[:, 1:15:2, :]
                )
                if k == TK - 1:
                    nc.vector.tensor_sub(
                        v[3][:, g0 : g0 + 6, :], xc[:, 1:13:2, :], xc[:, 3:15:2, :]
                    )
                    nc.vector.tensor_copy(v[3][:, G - 1 : G, :], xc[:, 13:14, :])
                else:
                    nc.vector.tensor_sub(
                        v[3][:, g0 : g0 + 7, :], xc[:, 1:15:2, :], xc[:, 3:16:2, :]
                    )

            # remaining input images: all prefetches queue on the scalar ring
            # BEHIND the startup-critical transfers (the Tile scheduler keeps
            # same-engine emission order for ready instructions, so these
            # can't steal startup DMA bandwidth). xipool bufs=3 lets all
            # three sit in SBUF from early on.
            nc.scalar.dma_start(out=bias_sb[:], in_=bt)
            xi_tiles = {}

            def load_xi(n, eng):
                xi = xipool.tile([C_IN, H, W], _DT, tag="xi", name=f"xi{n}")
                eng.dma_start(out=xi[:], in_=xs[n])
                xi_tiles[n] = xi

            for nn in range(1, N_PER):
                load_xi(nn, nc.scalar)

            def v_transform(n):
                xi = xi_tiles[n]
                v = [
                    vpool.tile([C_IN, G, W], _DT, tag="v", name=f"v{i}_{n}")
                    for i in range(4)
                ]
                nc.vector.tensor_sub(
                    v[0][:, 1:G, :], xi[:, 1:54:2, :], xi[:, 3:56:2, :]
                )
                nc.vector.tensor_scalar_mul(v[0][:, 0:1, :], xi[:, 1:2, :], -1.0)
                nc.vector.tensor_add(v[1][:], xi[:, 0:55:2, :], xi[:, 1:56:2, :])
                nc.vector.tensor_sub(v[2][:], xi[:, 1:56:2, :], xi[:, 0:55:2, :])
                nc.vector.tensor_sub(
                    v[3][:, 0 : G - 1, :], xi[:, 0:53:2, :], xi[:, 2:55:2, :]
                )
                nc.vector.tensor_copy(v[3][:, G - 1 : G, :], xi[:, 54:55, :])
                return v

            def as3(ap):
                return ap.rearrange("p (g w) -> p g w", g=GPT)

            for n in range(N_PER):
                v = v0tiles if n == 0 else v_transform(n)
                for k in range(TK):
                    g0 = 7 * k
                    last = n == N_PER - 1 and k == TK - 1
                    st = spool.tile([128, 2, 4, FD], _DT, tag="st", name=f"st_{n}_{k}")
                    for half in range(2):
                        m = pspool.tile(
                            [128, 4, 512], _F32, tag="ps", name=f"m_{n}_{k}_{half}"
                        )
                        for i in range(4):
                            mi = m[:, i, :FD].rearrange("p (g w) -> p g w", g=GPT)
                            for kw in _KW_ORDER:
                                if kw == 1:
                                    out_ap = mi
                                    rhs = v[i][:, g0 : g0 + 7, :]
                                elif kw == 0:
                                    out_ap = mi[:, :, 1:W]
                                    rhs = v[i][:, g0 : g0 + 7, 0 : W - 1]
                                else:
                                    out_ap = mi[:, :, 0 : W - 1]
                                    rhs = v[i][:, g0 : g0 + 7, 1:W]
                                nc.tensor.matmul(
                                    out_ap,
                                    lhsT=wslice(i, kw, half),
                                    rhs=rhs,
                                    start=(kw == 1),
                                    stop=(kw == 2),
                                )
                        if last:
                            # split the act so eviction overlaps the final
                            # matmuls and the drain chain is short
                            nc.scalar.activation(
                                st[:, half, 0:2], m[:, 0:2, :FD], _ID
                            )
                            nc.scalar.activation(
                                st[:, half, 2:4], m[:, 2:4, :FD], _ID
                            )
                        else:
                            # one act evicts all four comps (bf16, FD=1568)
                            nc.scalar.activation(st[:, half], m[:, :, :FD], _ID)

                    # output transform; p/u batched across both cout halves
                    # (per-half for the last tile to shorten the tail chain)
                    pb = qpool.tile([128, 2, FD], _DT, tag="pb", name=f"pb_{n}_{k}")
                    ub = qpool.tile([128, 2, FD], _DT, tag="ub", name=f"ub_{n}_{k}")
                    if last:
                        for half in range(2):
                            nc.vector.tensor_add(
                                pb[:, half], st[:, half, 1, :], st[:, half, 2, :]
                            )
                            nc.vector.tensor_sub(
                                ub[:, half], st[:, half, 1, :], st[:, half, 2, :]
                            )
                    else:
                        nc.vector.tensor_add(pb[:], st[:, :, 1, :], st[:, :, 2, :])
                        nc.vector.tensor_sub(ub[:], st[:, :, 1, :], st[:, :, 2, :])

                    for half in range(2):
                        bias_ap = bias_sb[:, half : half + 1]
                        yt = ypool.tile(
                            [128, 14, W], _F32, tag="yt", name=f"y_{n}_{k}_{half}"
                        )
                        nc.vector.scalar_tensor_tensor(
                            yt[:, 0:14:2, :],
                            as3(st[:, half, 0, :]),
                            bias_ap,
                            as3(pb[:, half, :]),
                            op0=_ADD,
                            op1=_ADD,
                        )
                        nc.vector.scalar_tensor_tensor(
                            yt[:, 1:14:2, :],
                            as3(ub[:, half, :]),
                            bias_ap,
                            as3(st[:, half, 3, :]),
                            op0=_ADD,
                            op1=_SUB,
                        )

                        y_slice = y[
                            n, half * 128 : (half + 1) * 128, 14 * k : 14 * k + 14, :
                        ]
                        if last and half == 1:
                            # split the final store so its DMA drain doesn't
                            # gate the end barrier on one queue
                            nc.sync.dma_start(out=y_slice[:, 0:7, :], in_=yt[:, 0:7, :])
                            nc.scalar.dma_start(
                                out=y_slice[:, 7:14, :], in_=yt[:, 7:14, :]
                            )
                        elif n == N_PER - 1 and half == 1:
                            # spread the last image's stores over two queues
                            # to avoid a terminal backlog on sync
                            nc.scalar.dma_start(out=y_slice, in_=yt[:])
                        else:
                            nc.sync.dma_start(out=y_slice, in_=yt[:])
    nc.compile()
    return nc


_NC = None


def _get_nc():
    global _NC
    if _NC is None:
        _NC = _build()
    return _NC


def _prep_inputs(x, kernels, b):
    bf16 = ml_dtypes.bfloat16
    xb = np.ascontiguousarray(x, dtype=np.float32).astype(bf16)
    w = np.asarray(kernels, dtype=np.float32)  # [O, C, kh, kw]
    U = [
        w[:, :, 0, :],
        0.5 * (w[:, :, 0, :] + w[:, :, 1, :] + w[:, :, 2, :]),
        0.5 * (w[:, :, 0, :] - w[:, :, 1, :] + w[:, :, 2, :]),
        w[:, :, 2, :],
    ]
    blocks = []
    for Ui in U:
        for kw in _KW_ORDER:
            blocks.append(Ui[:, :, kw].T)  # [C=128, O=256]
    wtb = np.ascontiguousarray(np.concatenate(blocks, axis=1)).astype(bf16)
    # bias [256] -> [128, 2]: column h holds b[h*128 : (h+1)*128]
    btb = np.ascontiguousarray(np.asarray(b, dtype=np.float32).reshape(2, 128).T)
    return xb, wtb, btb


def kernel(x, kernels, b):
    nc = _get_nc()
    xb, wtb, btb = _prep_inputs(x, kernels, b)
    in_maps = [
        {"xs": xb[i * N_PER : (i + 1) * N_PER], "wt": wtb, "bt": btb}
        for i in range(N_CORES)
    ]
    res = run_bass_kernel_spmd(nc, in_maps, core_ids=list(range(N_CORES)))
    out = np.concatenate(
        [r["y"].reshape(N_PER, C_OUT, H, W) for r in res.results], axis=0
    )
    return np.ascontiguousarray(out, dtype=np.float32)


# revision 16
# speedup vs baseline: 1.0977x; 1.0116x over previous
"""Conv2d 3x3 (stride 1, pad 1) Trainium2 Bass kernel — Winograd F(2,3) along H.

Problem: x (32, 128, 56, 56) fp32, kernels (256, 128, 3, 3) fp32, b (256,) fp32
-> out (32, 256, 56, 56) fp32.

Strategy:
  - Data-parallel over batch: 32 images / 8 cores = 4 images per core. SPMD,
    no collectives.
  - Winograd F(2,3) along H (row pairs): the 3 kh taps collapse into 4
    components m_i = sum_kw U[i,kw]^T V_i(col-shift kw), so each pair of
    output rows costs 12 matmuls instead of 18 direct taps (2/3 the PE work;
    direct conv is PE-roofline-bound at ~92us/core, Winograd ~62us).
      V0 = x[2g-1]-x[2g+1], V1 = x[2g]+x[2g+1], V2 = x[2g+1]-x[2g],
      V3 = x[2g]-x[2g+2]
      y[2g]   = m0+m1+m2+b,  y[2g+1] = m1-m2-m3+b
  - U = G-transform of the kernel along kh (host, fp32) stored bf16 as 12
    [128cin x 256cout] blocks ordered (comp, kw=1,0,2).
  - Tiling: 7 row-pair groups per tile; all 4 m_i of a tile-half live in ONE
    4-bank PSUM tile [128, 4, 512] so a single ScalarE activation evicts all
    of them (PSUM->SBUF bf16, FD=1568) — scalar stays under the PE's ~2us
    per tile-half. Column zero-pad is implicit via ragged kw=0/2 windows
    (kw=1 first with start=True).
  - Output transform on DVE only (GpSimd elementwise contends with DVE for
    the shared SBUF port — measured 3x slowdowns — so GpSimd only issues two
    early DMAs): p/u = s1+-s2 batched across both cout halves (bf16 2x),
    then per half y_even = (s0+b)+p and y_odd = (u+b)-s3 as fused
    scalar_tensor_tensor ops writing interleaved rows of the fp32 y tile.
  - V transform (DVE bf16 2x, row pairs as outer AP dims): whole-image ops
    for images 1..3 (x prefetched one image ahead); image 0 is chunked
    per-tile (16-row halo chunks) so the first matmul is gated only on a
    229KB DMA + one DVE op, with the weight DMAs interleaved between chunk
    loads on the scalar queue and 5 warm-up matmuls (~1.6us) bridging the
    HAM clock gate into the first real matmul.
"""

import numpy as np
import ml_dtypes

import concourse.bass as bass
import concourse.tile as tile
from concourse import bacc, mybir
from concourse.bass_utils import run_bass_kernel_spmd

N_CORES = 8
N_FULL = 32
N_PER = N_FULL // N_CORES  # 4 images per core
C_IN = 128
C_OUT = 256
H = W = 56
G = H // 2          # 28 row-pair groups
TK = 4              # m-tiles per image (along H)
GPT = G // TK       # 7 row-pair groups per tile
FD = GPT * W        # 392

_DT = mybir.dt.bfloat16
_F32 = mybir.dt.float32
_ID = mybir.ActivationFunctionType.Identity
_ADD = mybir.AluOpType.add
_SUB = mybir.AluOpType.subtract

_KW_ORDER = [1, 0, 2]
_KW_POS = {1: 0, 0: 1, 2: 2}


def _build():
    nc = bacc.Bacc(
        "TRN2",
        target_bir_lowering=False,
        debug=False,
        num_devices=N_CORES,
    )
    xs = nc.dram_tensor("xs", [N_PER, C_IN, H, W], _DT, kind="ExternalInput").ap()
    wt = nc.dram_tensor("wt", [C_IN, 12 * C_OUT], _DT, kind="ExternalInput").ap()
    bt = nc.dram_tensor("bt", [128, 2], _F32, kind="ExternalInput").ap()
    y = nc.dram_tensor("y", [N_PER, C_OUT, H, W], _F32, kind="ExternalOutput").ap()

    with tile.TileContext(nc) as tc:
        with (
            tc.tile_pool(name="const", bufs=1) as const,
            tc.tile_pool(name="xcpool", bufs=4) as xcpool,
            tc.tile_pool(name="xipool", bufs=3) as xipool,
            tc.tile_pool(name="vpool", bufs=8) as vpool,
            tc.tile_pool(name="spool", bufs=3) as spool,
            tc.tile_pool(name="qpool", bufs=6) as qpool,
            tc.tile_pool(name="ypool", bufs=4) as ypool,
            tc.tile_pool(name="pspool", bufs=2, space="PSUM") as pspool,
        ):
            # PE warm-up: ~1.6us of dummy matmuls during the input-load
            # window keep the HAM activity window busy so the clock gate
            # lifts (1.2 -> 2.4 GHz) right as the real matmuls begin.
            warm = const.tile([128, 512], _DT)
            nc.vector.memset(warm[:], 0.0)
            wps = pspool.tile([128, 4, 512], _F32, tag="ps", name="warm_ps")
            N_WARM = 5
            for i in range(N_WARM):
                nc.tensor.matmul(
                    wps[:, 0, :FD],
                    lhsT=warm[:, :128],
                    rhs=warm[:, :FD],
                    start=(i == 0),
                    stop=(i == N_WARM - 1),
                )

            bias_sb = const.tile([128, 2], _F32)
            wt_sb0 = const.tile([C_IN, 6 * C_OUT], _DT)
            wt_sb1 = const.tile([C_IN, 6 * C_OUT], _DT)

            def wslice(i, kw, half):
                bi = i * 3 + _KW_POS[kw]
                sb = wt_sb0 if bi < 6 else wt_sb1
                c0 = (bi % 6) * C_OUT + half * 128
                return sb[:, c0 : c0 + 128]

            # ---- image 0: chunked x + per-chunk V, weights interleaved ----
            v0tiles = [
                vpool.tile([C_IN, G, W], _DT, tag="v", name=f"v{i}_0")
                for i in range(4)
            ]
            xc_tiles = []
            for k in range(TK):
                lo = max(0, 14 * k - 1)
                hi = min(H, 14 * k + 15)
                loff = lo - (14 * k - 1)
                xc = xcpool.tile([C_IN, 16, W], _DT, tag="xc", name=f"xc{k}")
                nc.scalar.dma_start(
                    out=xc[:, loff : loff + (hi - lo), :], in_=xs[0, :, lo:hi, :]
                )
                xc_tiles.append(xc)
                if k == 0:
                    # weights ride the otherwise-idle sync + gpsimd queues so
                    # they land in parallel with chunk 0 on the scalar queue
                    nc.sync.dma_start(out=wt_sb0[:, : 3 * C_OUT], in_=wt[:, : 3 * C_OUT])
                    nc.sync.dma_start(
                        out=wt_sb0[:, 3 * C_OUT :], in_=wt[:, 3 * C_OUT : 6 * C_OUT]
                    )
                    nc.gpsimd.dma_start(out=wt_sb1[:], in_=wt[:, 6 * C_OUT :])

                v = v0tiles
                g0 = 7 * k
                if k == 0:
                    nc.vector.tensor_sub(
                        v[0][:, 1:7, :], xc[:, 2:14:2, :], xc[:, 4:16:2, :]
                    )
                    nc.vector.tensor_scalar_mul(v[0][:, 0:1, :], xc[:, 2:3, :], -1.0)
                else:
                    nc.vector.tensor_sub(
                        v[0][:, g0 : g0 + 7, :], xc[:, 0:14:2, :], xc[:, 2:16:2, :]
                    )
                nc.vector.tensor_add(
                    v[1][:, g0 : g0 + 7, :], xc[:, 1:15:2, :], xc[:, 2:16:2, :]
                )
                nc.vector.tensor_sub(
                    v[2][:, g0 : g0 + 7, :], xc[:, 2:16:2, :], xc[:, 1:15:2, :]
                )
                if k == TK - 1:
                    nc.vector.tensor_sub(
                        v[3][:, g0 : g0 + 6, :], xc[:, 1:13:2, :], xc[:, 3:15:2, :]
                    )
                    nc.vector.tensor_copy(v[3][:, G - 1 : G, :], xc[:, 13:14, :])
                else:
                    nc.vector.tensor_sub(
                        v[3][:, g0 : g0 + 7, :], xc[:, 1:15:2, :], xc[:, 3:16:2, :]
                    )

            # remaining input images: all prefetches queue on the scalar ring
            # BEHIND the startup-critical transfers (the Tile scheduler keeps
            # same-engine emission order for ready instructions, so these
            # can't steal startup DMA bandwidth). xipool bufs=3 lets all
            # three sit in SBUF from early on.
            nc.scalar.dma_start(out=bias_sb[:], in_=bt)
            xi_tiles = {}

            def load_xi(n, eng):
                xi = xipool.tile([C_IN, H, W], _DT, tag="xi", name=f"xi{n}")
                eng.dma_start(out=xi[:], in_=xs[n])
                xi_tiles[n] = xi

            for nn in range(1, N_PER):
                load_xi(nn, nc.scalar)

            def v_transform(n):
                xi = xi_tiles[n]
                v = [
                    vpool.tile([C_IN, G, W], _DT, tag="v", name=f"v{i}_{n}")
                    for i in range(4)
                ]
                nc.vector.tensor_sub(
                    v[0][:, 1:G, :], xi[:, 1:54:2, :], xi[:, 3:56:2, :]
                )
                nc.vector.tensor_scalar_mul(v[0][:, 0:1, :], xi[:, 1:2, :], -1.0)
                nc.vector.tensor_add(v[1][:], xi[:, 0:55:2, :], xi[:, 1:56:2, :])
                nc.vector.tensor_sub(v[2][:], xi[:, 1:56:2, :], xi[:, 0:55:2, :])
                nc.vector.tensor_sub(
                    v[3][:, 0 : G - 1, :], xi[:, 0:53:2, :], xi[:, 2:55:2, :]
                )
                nc.vector.tensor_copy(v[3][:, G - 1 : G, :], xi[:, 54:55, :])
                return v

            def as3(ap):
                return ap.rearrange("p (g w) -> p g w", g=GPT)

            for n in range(N_PER):
                v = v0tiles if n == 0 else v_transform(n)
                if n < N_PER - 1:
                    segs = [(7 * k, 7) for k in range(TK)]
                else:
                    # the last image ends with two short segments so the
                    # terminal eviction + store chain after the final matmul
                    # is ~2x shorter
                    segs = [(0, 7), (7, 7), (14, 7), (21, 4), (25, 3)]
                for si, (g0, gn) in enumerate(segs):
                    fdp = gn * W
                    last = n == N_PER - 1 and si == len(segs) - 1
                    st = spool.tile([128, 2, 4, FD], _DT, tag="st", name=f"st_{n}_{si}")
                    for half in range(2):
                        m = pspool.tile(
                            [128, 4, 512], _F32, tag="ps", name=f"m_{n}_{si}_{half}"
                        )
                        for i in range(4):
                            mi = m[:, i, :fdp].rearrange("p (g w) -> p g w", g=gn)
                            for kw in _KW_ORDER:
                                if kw == 1:
                                    out_ap = mi
                                    rhs = v[i][:, g0 : g0 + gn, :]
                                elif kw == 0:
                                    out_ap = mi[:, :, 1:W]
                                    rhs = v[i][:, g0 : g0 + gn, 0 : W - 1]
                                else:
                                    out_ap = mi[:, :, 0 : W - 1]
                                    rhs = v[i][:, g0 : g0 + gn, 1:W]
                                nc.tensor.matmul(
                                    out_ap,
                                    lhsT=wslice(i, kw, half),
                                    rhs=rhs,
                                    start=(kw == 1),
                                    stop=(kw == 2),
                                )
                        # one act evicts all four comps (bf16)
                        nc.scalar.activation(
                            st[:, half, :, :fdp], m[:, :, :fdp], _ID
                        )

                    # output transform; p/u batched across both cout halves
                    pb = qpool.tile([128, 2, FD], _DT, tag="pb", name=f"pb_{n}_{si}")
                    nc.vector.tensor_add(
                        pb[:, :, :fdp], st[:, :, 1, :fdp], st[:, :, 2, :fdp]
                    )
                    ub = qpool.tile([128, 2, FD], _DT, tag="ub", name=f"ub_{n}_{si}")
                    nc.vector.tensor_sub(
                        ub[:, :, :fdp], st[:, :, 1, :fdp], st[:, :, 2, :fdp]
                    )

                    for half in range(2):
                        bias_ap = bias_sb[:, half : half + 1]

                        def as3(ap, gn=gn):
                            return ap.rearrange("p (g w) -> p g w", g=gn)

                        yt = ypool.tile(
                            [128, 14, W], _F32, tag="yt", name=f"y_{n}_{si}_{half}"
                        )
                        nc.vector.scalar_tensor_tensor(
                            yt[:, 0 : 2 * gn : 2, :],
                            as3(st[:, half, 0, :fdp]),
                            bias_ap,
                            as3(pb[:, half, :fdp]),
                            op0=_ADD,
                            op1=_ADD,
                        )
                        nc.vector.scalar_tensor_tensor(
                            yt[:, 1 : 2 * gn : 2, :],
                            as3(ub[:, half, :fdp]),
                            bias_ap,
                            as3(st[:, half, 3, :fdp]),
                            op0=_ADD,
                            op1=_SUB,
                        )

                        r0 = 2 * g0
                        y_slice = y[
                            n, half * 128 : (half + 1) * 128, r0 : r0 + 2 * gn, :
                        ]
                        if last and half == 1:
                            # split the final store so its DMA drain doesn't
                            # gate the end barrier on one queue
                            nc.sync.dma_start(
                                out=y_slice[:, 0:gn, :], in_=yt[:, 0:gn, :]
                            )
                            nc.scalar.dma_start(
                                out=y_slice[:, gn : 2 * gn, :],
                                in_=yt[:, gn : 2 * gn, :],
                            )
                        elif n == N_PER - 1 and half == 1 and not last:
                            # spread the last image's stores over two queues
                            # to avoid a terminal backlog on sync
                            nc.scalar.dma_start(
                                out=y_slice, in_=yt[:, : 2 * gn, :]
                            )
                        else:
                            nc.sync.dma_start(out=y_slice, in_=yt[:, : 2 * gn, :])
    nc.compile()
    return nc


_NC = None


def _get_nc():
    global _NC
    if _NC is None:
        _NC = _build()
    return _NC


def _prep_inputs(x, kernels, b):
    bf16 = ml_dtypes.bfloat16
    xb = np.ascontiguousarray(x, dtype=np.float32).astype(bf16)
    w = np.asarray(kernels, dtype=np.float32)  # [O, C, kh, kw]
    U = [
        w[:, :, 0, :],
        0.5 * (w[:, :, 0, :] + w[:, :, 1, :] + w[:, :, 2, :]),
        0.5 * (w[:, :, 0, :] - w[:, :, 1, :] + w[:, :, 2, :]),
        w[:, :, 2, :],
    ]
    blocks = []
    for Ui in U:
        for kw in _KW_ORDER:
            blocks.append(Ui[:, :, kw].T)  # [C=128, O=256]
    wtb = np.ascontiguousarray(np.concatenate(blocks, axis=1)).astype(bf16)
    # bias [256] -> [128, 2]: column h holds b[h*128 : (h+1)*128]
    btb = np.ascontiguousarray(np.asarray(b, dtype=np.float32).reshape(2, 128).T)
    return xb, wtb, btb


def kernel(x, kernels, b):
    nc = _get_nc()
    xb, wtb, btb = _prep_inputs(x, kernels, b)
    in_maps = [
        {"xs": xb[i * N_PER : (i + 1) * N_PER], "wt": wtb, "bt": btb}
        for i in range(N_CORES)
    ]
    res = run_bass_kernel_spmd(nc, in_maps, core_ids=list(range(N_CORES)))
    out = np.concatenate(
        [r["y"].reshape(N_PER, C_OUT, H, W) for r in res.results], axis=0
    )
    return np.ascontiguousarray(out, dtype=np.float32)
